# revision 12
# baseline (speedup 1.0000x reference)
"""Trainium2 Bass kernel for CellPathwayAttentionAggregator (segment-reduce).

Math: out[b, s] = sum_{i in set s} softmax_s(attn_logits)[i] * G[b, flat_idx[i]]

Device decomposition (per core):
    out = (G @ W_exp) * (1 / denom)[None, :]
where W_exp[g, s] = sum_{i in set s, flat_idx[i]=g} exp(attn_logits[i]) is the
(unnormalized) sparse aggregation matrix, scattered on the host as pure layout
prep (elementwise exp + scatter; no reductions on host), and
    denom[s] = sum_{i in set s} exp(attn_logits[i])
is computed ON DEVICE from a 128-slot padded logits tile (ACT exp + ones-vector
matmul), followed by on-device normalization of the matmul output.

Sharding: 8 cores = 2 batch groups (512 rows) x 4 set groups (512 sets).
Each core runs a (512 x 8192) @ (8192 x 512) bf16 matmul accumulated in fp32
PSUM over 64 K-tiles, then scales each output column by 1/denom.
"""

import sys

if "/opt/trn_rl_repo" not in sys.path:
    sys.path.insert(0, "/opt/trn_rl_repo")

import ml_dtypes
import numpy as np

NUM_SETS = 2048
NUM_GENESETS = 8192
BATCH = 1024
N_CORES = 8
BG, SG = 2, 4  # batch groups x set groups (BG*SG == N_CORES)
B_C = BATCH // BG  # 512 batch rows per core
S_C = NUM_SETS // SG  # 512 sets per core
P = 128
K_TILES = NUM_GENESETS // P  # 64
M_TILES = B_C // P  # 4
PAD_SLOTS = 128  # >= MAX set size (120)
NEG_FILL = -87.0  # exp(-87) ~ 1.6e-38 ~ 0 in fp32

_PROGRAM_CACHE = {}
LAST_RESULTS = None  # BassKernelResults of the most recent run (for profiling)


def _build_program():
    import concourse.mybir as mybir
    from concourse import bacc
    from concourse.tile import TileContext

    f32 = mybir.dt.float32
    bf16 = mybir.dt.bfloat16

    nc = bacc.Bacc("TRN2", target_bir_lowering=False, debug=False)
    # fused per-K-tile input: [:, :, :B_C] = G^T tile, [:, :, B_C:] = W tile.
    # One DMA per K-tile keeps every matmul's sync-wait count at <=1 (the
    # S3 LDWEIGHTS encoding only has a single wait slot).
    gw_d = nc.dram_tensor("gw", [K_TILES, P, B_C + S_C], bf16, kind="ExternalInput")
    plog_d = nc.dram_tensor("plog", [PAD_SLOTS, S_C], f32, kind="ExternalInput")
    out_d = nc.dram_tensor("out", [B_C, S_C], f32, kind="ExternalOutput")

    with TileContext(nc) as tc:
        with (
            tc.tile_pool(name="const", bufs=1) as cpool,
            tc.tile_pool(name="gw", bufs=6) as gwpool,
            tc.tile_pool(name="outp", bufs=2) as opool,
            tc.tile_pool(name="ps", bufs=1, space="PSUM") as ppool,
        ):
            # --- inputs for the denominator chain (SWDGE so it doesn't queue
            # behind the gw HWDGE stream) ---
            plog_sb = cpool.tile([PAD_SLOTS, S_C], f32, tag="plog")
            nc.gpsimd.dma_start(out=plog_sb[:], in_=plog_d[:, :])
            exp_sb = cpool.tile([PAD_SLOTS, S_C], f32, tag="exp")
            nc.scalar.activation(
                exp_sb[:], plog_sb[:], mybir.ActivationFunctionType.Exp
            )
            # ones vector built on ACT so the denom matmul waits on one engine
            ones_col = cpool.tile([P, 1], f32, tag="onec")
            nc.scalar.activation(
                ones_col[:],
                plog_sb[:, 0:1],
                mybir.ActivationFunctionType.Copy,
                bias=1.0,
                scale=0.0,
            )
            ones_row = cpool.tile([1, P], f32, tag="oner")
            nc.vector.memset(ones_row[:], 1.0)

            # --- PE warmup: ~3.5us of dummy matmuls on a memset tile keeps the
            # HAM clock-gate busy while the first gw DMA lands, so the real
            # stream starts at 2.4 GHz instead of 1.2 GHz.
            warm_sb = cpool.tile([P, S_C], bf16, tag="warm")
            nc.vector.memset(warm_sb[:], 0.0)
            scratch_ps = ppool.tile([P, S_C], f32, tag="scratch")
            for _ in range(16):
                nc.tensor.matmul(
                    scratch_ps[:],
                    warm_sb[:, :P],
                    warm_sb[:],
                    start=True,
                    stop=True,
                )

            # --- main matmul: out = G_c @ W_c, accumulated over 64 K-tiles ---
            acc = [
                ppool.tile([P, S_C], f32, tag=f"acc{m}", name=f"acc{m}")
                for m in range(M_TILES)
            ]
            denom_ps = ppool.tile([1, S_C], f32, tag="denom")
            recip_sb = cpool.tile([1, S_C], f32, tag="recip")
            rep_ps = ppool.tile([P, S_C], f32, tag="rep")
            recip_rep = cpool.tile([P, S_C], f32, tag="recrep")
            for k in range(K_TILES):
                gw_sb = gwpool.tile([P, B_C + S_C], bf16, tag="gw")
                nc.sync.dma_start(out=gw_sb[:], in_=gw_d[k, :, :])
                for m in range(M_TILES):
                    nc.tensor.matmul(
                        acc[m][:],
                        gw_sb[:, m * P : (m + 1) * P],
                        gw_sb[:, B_C : B_C + S_C],
                        start=(k == 0),
                        stop=(k == K_TILES - 1),
                    )
                # denominator + replication chain injected mid-stream so the
                # reciprocal is ready long before the epilogue
                if k == 8:
                    nc.tensor.matmul(
                        denom_ps[:], ones_col[:], exp_sb[:], start=True, stop=True
                    )
                elif k == 9:
                    nc.vector.reciprocal(recip_sb[:], denom_ps[:])
                elif k == 12:
                    nc.tensor.matmul(
                        rep_ps[:], ones_row[:], recip_sb[:], start=True, stop=True
                    )
                elif k == 13:
                    nc.vector.tensor_copy(recip_rep[:], rep_ps[:])

            # --- normalize each output column by 1/denom and store ---
            for m in range(M_TILES):
                o_sb = opool.tile([P, S_C], f32, tag="osb")
                nc.vector.tensor_mul(o_sb[:], acc[m][:], recip_rep[:])
                nc.sync.dma_start(out=out_d[m * P : (m + 1) * P, :], in_=o_sb[:])

    nc.finalize()
    return nc


def _get_program():
    if "nc" not in _PROGRAM_CACHE:
        _PROGRAM_CACHE["nc"] = _build_program()
    return _PROGRAM_CACHE["nc"]


def _ensure_ntff_hook():
    """Make NTFF profiling under axon work (BASS_TRACE=1): the image's antenv
    package lacks the axon_hooks holder module, so synthesize it and register
    the ctypes-based profile hook from trn_agent_boot. Best-effort."""
    import types

    try:
        import antenv

        try:
            from antenv.axon_hooks import get_axon_ntff_profile_hook  # noqa: F401

            return  # already present and registered
        except ImportError:
            pass
        mod = types.ModuleType("antenv.axon_hooks")
        _holder = [None]
        mod.set_axon_ntff_profile_hook = lambda h: _holder.__setitem__(0, h)
        mod.get_axon_ntff_profile_hook = lambda: _holder[0]
        sys.modules["antenv.axon_hooks"] = mod
        antenv.axon_hooks = mod

        from trn_agent_boot.trn_boot import _ntff_profile_via_ctypes

        hook = _ntff_profile_via_ctypes("/opt/axon/libaxon_pjrt.so")
        mod.set_axon_ntff_profile_hook(hook)
    except Exception:
        pass


def kernel(**inputs):
    global LAST_RESULTS
    G = np.asarray(inputs["geneset_features"], dtype=np.float32)
    logits = np.asarray(inputs["attn_logits"], dtype=np.float32)
    flat_idx = np.asarray(inputs["flat_idx"]).astype(np.int64)
    seg = np.asarray(inputs["segment_ids"]).astype(np.int64)
    T = logits.shape[0]

    # Host-side layout prep: scatter exp(logits) into the sparse aggregation
    # matrix (member sets are sampled without replacement, so (idx, seg) pairs
    # are unique within a set and the fancy assignment is collision-free).
    e32 = np.exp(logits)
    W = np.zeros((NUM_GENESETS, NUM_SETS), dtype=ml_dtypes.bfloat16)
    W[flat_idx, seg] = e32.astype(ml_dtypes.bfloat16)

    # Padded per-set logit columns; device computes denominators from these.
    sizes = np.bincount(seg, minlength=NUM_SETS)
    starts = np.concatenate([[0], np.cumsum(sizes)[:-1]])
    pos = np.arange(T) - starts[seg]
    plogT = np.full((PAD_SLOTS, NUM_SETS), NEG_FILL, dtype=np.float32)
    plogT[pos, seg] = logits

    Gb = G.astype(ml_dtypes.bfloat16)

    GbT = np.ascontiguousarray(Gb.T)  # (8192, 1024)
    in_maps = []
    for c in range(N_CORES):
        bg, sg = divmod(c, SG)
        gt = GbT[:, bg * B_C : (bg + 1) * B_C].reshape(K_TILES, P, B_C)
        w = W[:, sg * S_C : (sg + 1) * S_C].reshape(K_TILES, P, S_C)
        gw = np.concatenate([gt, w], axis=2)  # (K_TILES, P, B_C + S_C)
        plog = np.ascontiguousarray(plogT[:, sg * S_C : (sg + 1) * S_C])
        in_maps.append({"gw": np.ascontiguousarray(gw), "plog": plog})

    from concourse.bass_utils import run_bass_kernel_spmd

    _ensure_ntff_hook()
    nc = _get_program()
    res = run_bass_kernel_spmd(nc, in_maps, core_ids=list(range(N_CORES)))
    LAST_RESULTS = res

    out = np.empty((BATCH, NUM_SETS), dtype=np.float32)
    for c in range(N_CORES):
        bg, sg = divmod(c, SG)
        out[bg * B_C : (bg + 1) * B_C, sg * S_C : (sg + 1) * S_C] = res.results[c][
            "out"
        ]
    return out


# revision 14
# speedup vs baseline: 1.1369x; 1.1369x over previous
"""Trainium2 Bass kernel for CellPathwayAttentionAggregator (segment-reduce).

Math: out[b, s] = sum_{i in set s} softmax_s(attn_logits)[i] * G[b, flat_idx[i]]

Device decomposition (per core):
    out = (G @ W_exp) * (1 / denom)[None, :]
where W_exp[g, s] = sum_{i in set s, flat_idx[i]=g} exp(attn_logits[i]) is the
(unnormalized) sparse aggregation matrix, scattered on the host as pure layout
prep (elementwise exp + scatter; no reductions on host), and
    denom[s] = sum_{i in set s} exp(attn_logits[i])
is computed ON DEVICE from a 128-slot padded logits tile (ACT exp + ones-vector
matmul), followed by on-device normalization of the matmul output.

Sharding: 8 cores = 2 batch groups (512 rows) x 4 set groups (512 sets).
Each core runs a (512 x 8192) @ (8192 x 512) bf16 matmul accumulated in fp32
PSUM over 64 K-tiles, then scales each output column by 1/denom.
"""

import sys

if "/opt/trn_rl_repo" not in sys.path:
    sys.path.insert(0, "/opt/trn_rl_repo")

import ml_dtypes
import numpy as np

NUM_SETS = 2048
NUM_GENESETS = 8192
BATCH = 1024
N_CORES = 8
BG, SG = 2, 4  # batch groups x set groups (BG*SG == N_CORES)
B_C = BATCH // BG  # 512 batch rows per core
S_C = NUM_SETS // SG  # 512 sets per core
P = 128
K_TILES = NUM_GENESETS // P  # 64
M_TILES = B_C // P  # 4
PAD_SLOTS = 128  # >= MAX set size (120)
NEG_FILL = -87.0  # exp(-87) ~ 1.6e-38 ~ 0 in fp32

_PROGRAM_CACHE = {}
LAST_RESULTS = None  # BassKernelResults of the most recent run (for profiling)


def _build_program():
    import concourse.mybir as mybir
    from concourse import bacc
    from concourse.tile import TileContext

    f32 = mybir.dt.float32
    bf16 = mybir.dt.bfloat16

    nc = bacc.Bacc("TRN2", target_bir_lowering=False, debug=False)
    # fused per-K-tile input: [:, :, :B_C] = G^T tile, [:, :, B_C:] = W tile.
    # One DMA per K-tile keeps every matmul's sync-wait count at <=1 (the
    # S3 LDWEIGHTS encoding only has a single wait slot).
    gw_d = nc.dram_tensor("gw", [K_TILES, P, B_C + S_C], bf16, kind="ExternalInput")
    plog_d = nc.dram_tensor("plog", [PAD_SLOTS, S_C], f32, kind="ExternalInput")
    out_d = nc.dram_tensor("out", [B_C, S_C], f32, kind="ExternalOutput")

    with TileContext(nc) as tc:
        with (
            tc.tile_pool(name="const", bufs=1) as cpool,
            tc.tile_pool(name="gw", bufs=12) as gwpool,
            tc.tile_pool(name="outp", bufs=4) as opool,
            tc.tile_pool(name="ps", bufs=1, space="PSUM") as ppool,
        ):
            # --- inputs for the denominator chain (SWDGE so it doesn't queue
            # behind the gw HWDGE stream) ---
            plog_sb = cpool.tile([PAD_SLOTS, S_C], f32, tag="plog")
            nc.gpsimd.dma_start(out=plog_sb[:], in_=plog_d[:, :])
            exp_sb = cpool.tile([PAD_SLOTS, S_C], f32, tag="exp")
            nc.scalar.activation(
                exp_sb[:], plog_sb[:], mybir.ActivationFunctionType.Exp
            )
            # ones vector built on ACT so the denom matmul waits on one engine
            ones_col = cpool.tile([P, 1], f32, tag="onec")
            nc.scalar.activation(
                ones_col[:],
                plog_sb[:, 0:1],
                mybir.ActivationFunctionType.Copy,
                bias=1.0,
                scale=0.0,
            )
            ones_row = cpool.tile([1, P], f32, tag="oner")
            nc.vector.memset(ones_row[:], 1.0)

            # --- main matmul: out = G_c @ W_c, accumulated over 64 K-tiles ---
            acc = [
                ppool.tile([P, S_C], f32, tag=f"acc{m}", name=f"acc{m}")
                for m in range(M_TILES)
            ]
            denom_ps = ppool.tile([1, S_C], f32, tag="denom")
            recip_sb = cpool.tile([1, S_C], f32, tag="recip")
            rep_ps = ppool.tile([P, S_C], f32, tag="rep")
            recip_rep = cpool.tile([P, S_C], f32, tag="recrep")
            for k in range(K_TILES):
                gw_sb = gwpool.tile([P, B_C + S_C], bf16, tag="gw")
                nc.sync.dma_start(out=gw_sb[:], in_=gw_d[k, :, :])
                for m in range(M_TILES):
                    nc.tensor.matmul(
                        acc[m][:],
                        gw_sb[:, m * P : (m + 1) * P],
                        gw_sb[:, B_C : B_C + S_C],
                        start=(k == 0),
                        stop=(k == K_TILES - 1),
                    )
                # denominator + replication chain injected mid-stream so the
                # reciprocal is ready long before the epilogue
                if k == 8:
                    nc.tensor.matmul(
                        denom_ps[:], ones_col[:], exp_sb[:], start=True, stop=True
                    )
                elif k == 9:
                    nc.vector.reciprocal(recip_sb[:], denom_ps[:])
                elif k == 12:
                    nc.tensor.matmul(
                        rep_ps[:], ones_row[:], recip_sb[:], start=True, stop=True
                    )
                elif k == 13:
                    nc.vector.tensor_copy(recip_rep[:], rep_ps[:])

            # --- normalize each output column by 1/denom and store ---
            for m in range(M_TILES):
                o_sb = opool.tile([P, S_C], f32, tag="osb")
                nc.vector.tensor_mul(o_sb[:], acc[m][:], recip_rep[:])
                nc.sync.dma_start(out=out_d[m * P : (m + 1) * P, :], in_=o_sb[:])

    nc.finalize()
    return nc


def _get_program():
    if "nc" not in _PROGRAM_CACHE:
        _PROGRAM_CACHE["nc"] = _build_program()
    return _PROGRAM_CACHE["nc"]


def _ensure_ntff_hook():
    """Make NTFF profiling under axon work (BASS_TRACE=1): the image's antenv
    package lacks the axon_hooks holder module, so synthesize it and register
    the ctypes-based profile hook from trn_agent_boot. Best-effort."""
    import types

    try:
        import antenv

        try:
            from antenv.axon_hooks import get_axon_ntff_profile_hook  # noqa: F401

            return  # already present and registered
        except ImportError:
            pass
        mod = types.ModuleType("antenv.axon_hooks")
        _holder = [None]
        mod.set_axon_ntff_profile_hook = lambda h: _holder.__setitem__(0, h)
        mod.get_axon_ntff_profile_hook = lambda: _holder[0]
        sys.modules["antenv.axon_hooks"] = mod
        antenv.axon_hooks = mod

        from trn_agent_boot.trn_boot import _ntff_profile_via_ctypes

        hook = _ntff_profile_via_ctypes("/opt/axon/libaxon_pjrt.so")
        mod.set_axon_ntff_profile_hook(hook)
    except Exception:
        pass


def kernel(**inputs):
    global LAST_RESULTS
    G = np.asarray(inputs["geneset_features"], dtype=np.float32)
    logits = np.asarray(inputs["attn_logits"], dtype=np.float32)
    flat_idx = np.asarray(inputs["flat_idx"]).astype(np.int64)
    seg = np.asarray(inputs["segment_ids"]).astype(np.int64)
    T = logits.shape[0]

    # Host-side layout prep: scatter exp(logits) into the sparse aggregation
    # matrix (member sets are sampled without replacement, so (idx, seg) pairs
    # are unique within a set and the fancy assignment is collision-free).
    e32 = np.exp(logits)
    W = np.zeros((NUM_GENESETS, NUM_SETS), dtype=ml_dtypes.bfloat16)
    W[flat_idx, seg] = e32.astype(ml_dtypes.bfloat16)

    # Padded per-set logit columns; device computes denominators from these.
    sizes = np.bincount(seg, minlength=NUM_SETS)
    starts = np.concatenate([[0], np.cumsum(sizes)[:-1]])
    pos = np.arange(T) - starts[seg]
    plogT = np.full((PAD_SLOTS, NUM_SETS), NEG_FILL, dtype=np.float32)
    plogT[pos, seg] = logits

    Gb = G.astype(ml_dtypes.bfloat16)

    GbT = np.ascontiguousarray(Gb.T)  # (8192, 1024)
    in_maps = []
    for c in range(N_CORES):
        bg, sg = divmod(c, SG)
        gt = GbT[:, bg * B_C : (bg + 1) * B_C].reshape(K_TILES, P, B_C)
        w = W[:, sg * S_C : (sg + 1) * S_C].reshape(K_TILES, P, S_C)
        gw = np.concatenate([gt, w], axis=2)  # (K_TILES, P, B_C + S_C)
        plog = np.ascontiguousarray(plogT[:, sg * S_C : (sg + 1) * S_C])
        in_maps.append({"gw": np.ascontiguousarray(gw), "plog": plog})

    from concourse.bass_utils import run_bass_kernel_spmd

    _ensure_ntff_hook()
    nc = _get_program()
    res = run_bass_kernel_spmd(nc, in_maps, core_ids=list(range(N_CORES)))
    LAST_RESULTS = res

    out = np.empty((BATCH, NUM_SETS), dtype=np.float32)
    for c in range(N_CORES):
        bg, sg = divmod(c, SG)
        out[bg * B_C : (bg + 1) * B_C, sg * S_C : (sg + 1) * S_C] = res.results[c][
            "out"
        ]
    return out


# revision 15
# speedup vs baseline: 1.1425x; 1.0049x over previous
"""Trainium2 Bass kernel for CellPathwayAttentionAggregator (segment-reduce).

Math: out[b, s] = sum_{i in set s} softmax_s(attn_logits)[i] * G[b, flat_idx[i]]

Device decomposition (per core):
    out = (G @ W_exp) * (1 / denom)[None, :]
where W_exp[g, s] = sum_{i in set s, flat_idx[i]=g} exp(attn_logits[i]) is the
(unnormalized) sparse aggregation matrix, scattered on the host as pure layout
prep (elementwise exp + scatter; no reductions on host), and
    denom[s] = sum_{i in set s} exp(attn_logits[i])
is computed ON DEVICE from a 128-slot padded logits tile (ACT exp + ones-vector
matmul), followed by on-device normalization of the matmul output.

Sharding: 8 cores = 2 batch groups (512 rows) x 4 set groups (512 sets).
Each core runs a (512 x 8192) @ (8192 x 512) bf16 matmul accumulated in fp32
PSUM over 64 K-tiles, then scales each output column by 1/denom.
"""

import sys

if "/opt/trn_rl_repo" not in sys.path:
    sys.path.insert(0, "/opt/trn_rl_repo")

import ml_dtypes
import numpy as np

NUM_SETS = 2048
NUM_GENESETS = 8192
BATCH = 1024
N_CORES = 8
BG, SG = 2, 4  # batch groups x set groups (BG*SG == N_CORES)
B_C = BATCH // BG  # 512 batch rows per core
S_C = NUM_SETS // SG  # 512 sets per core
P = 128
K_TILES = NUM_GENESETS // P  # 64
M_TILES = B_C // P  # 4
PAD_SLOTS = 128  # >= MAX set size (120)
NEG_FILL = -87.0  # exp(-87) ~ 1.6e-38 ~ 0 in fp32

_PROGRAM_CACHE = {}
LAST_RESULTS = None  # BassKernelResults of the most recent run (for profiling)


def _build_program():
    import concourse.mybir as mybir
    from concourse import bacc
    from concourse.tile import TileContext

    f32 = mybir.dt.float32
    bf16 = mybir.dt.bfloat16

    nc = bacc.Bacc("TRN2", target_bir_lowering=False, debug=False)
    # fused per-K-tile input: [:, :, :B_C] = G^T tile, [:, :, B_C:] = W tile.
    # One DMA per K-tile keeps every matmul's sync-wait count at <=1 (the
    # S3 LDWEIGHTS encoding only has a single wait slot).
    gw_d = nc.dram_tensor("gw", [K_TILES, P, B_C + S_C], bf16, kind="ExternalInput")
    plog_d = nc.dram_tensor("plog", [PAD_SLOTS, S_C], f32, kind="ExternalInput")
    out_d = nc.dram_tensor("out", [B_C, S_C], f32, kind="ExternalOutput")

    with TileContext(nc) as tc:
        with (
            tc.tile_pool(name="const", bufs=1) as cpool,
            tc.tile_pool(name="gw", bufs=12) as gwpool,
            tc.tile_pool(name="outp", bufs=4) as opool,
            tc.tile_pool(name="ps", bufs=1, space="PSUM") as ppool,
        ):
            # --- inputs for the denominator chain (SWDGE so it doesn't queue
            # behind the gw HWDGE stream) ---
            plog_sb = cpool.tile([PAD_SLOTS, S_C], f32, tag="plog")
            nc.gpsimd.dma_start(out=plog_sb[:], in_=plog_d[:, :])
            exp_sb = cpool.tile([PAD_SLOTS, S_C], f32, tag="exp")
            nc.scalar.activation(
                exp_sb[:], plog_sb[:], mybir.ActivationFunctionType.Exp
            )
            # ones vector built on ACT so the denom matmul waits on one engine
            ones_col = cpool.tile([P, 1], f32, tag="onec")
            nc.scalar.activation(
                ones_col[:],
                plog_sb[:, 0:1],
                mybir.ActivationFunctionType.Copy,
                bias=1.0,
                scale=0.0,
            )
            ones_row = cpool.tile([1, P], f32, tag="oner")
            nc.vector.memset(ones_row[:], 1.0)

            # --- PE warmup: short N=128 matmuls on a GPSIMD-memset tile run
            # while the first gw DMA is in flight, so the HAM clock-gate is
            # already at 8/8 (2.4 GHz) when the real stream starts.
            warm_sb = cpool.tile([P, P], bf16, tag="warm")
            nc.gpsimd.memset(warm_sb[:], 0.0)
            scratch_ps = ppool.tile([P, P], f32, tag="scratch")
            for _ in range(16):
                nc.tensor.matmul(
                    scratch_ps[:], warm_sb[:], warm_sb[:], start=True, stop=True
                )

            # --- main matmul: out = G_c @ W_c, accumulated over 64 K-tiles ---
            acc = [
                ppool.tile([P, S_C], f32, tag=f"acc{m}", name=f"acc{m}")
                for m in range(M_TILES)
            ]
            denom_ps = ppool.tile([1, S_C], f32, tag="denom")
            recip_sb = cpool.tile([1, S_C], f32, tag="recip")
            rep_ps = ppool.tile([P, S_C], f32, tag="rep")
            recip_rep = cpool.tile([P, S_C], f32, tag="recrep")
            for k in range(K_TILES):
                gw_sb = gwpool.tile([P, B_C + S_C], bf16, tag="gw")
                nc.sync.dma_start(out=gw_sb[:], in_=gw_d[k, :, :])
                for m in range(M_TILES):
                    nc.tensor.matmul(
                        acc[m][:],
                        gw_sb[:, m * P : (m + 1) * P],
                        gw_sb[:, B_C : B_C + S_C],
                        start=(k == 0),
                        stop=(k == K_TILES - 1),
                    )
                # denominator + replication chain injected mid-stream so the
                # reciprocal is ready long before the epilogue
                if k == 8:
                    nc.tensor.matmul(
                        denom_ps[:], ones_col[:], exp_sb[:], start=True, stop=True
                    )
                elif k == 9:
                    nc.vector.reciprocal(recip_sb[:], denom_ps[:])
                elif k == 12:
                    nc.tensor.matmul(
                        rep_ps[:], ones_row[:], recip_sb[:], start=True, stop=True
                    )
                elif k == 13:
                    nc.vector.tensor_copy(recip_rep[:], rep_ps[:])

            # --- normalize each output column by 1/denom and store ---
            for m in range(M_TILES):
                o_sb = opool.tile([P, S_C], f32, tag="osb")
                nc.vector.tensor_mul(o_sb[:], acc[m][:], recip_rep[:])
                nc.sync.dma_start(out=out_d[m * P : (m + 1) * P, :], in_=o_sb[:])

    nc.finalize()
    return nc


def _get_program():
    if "nc" not in _PROGRAM_CACHE:
        _PROGRAM_CACHE["nc"] = _build_program()
    return _PROGRAM_CACHE["nc"]


def _ensure_ntff_hook():
    """Make NTFF profiling under axon work (BASS_TRACE=1): the image's antenv
    package lacks the axon_hooks holder module, so synthesize it and register
    the ctypes-based profile hook from trn_agent_boot. Best-effort."""
    import types

    try:
        import antenv

        try:
            from antenv.axon_hooks import get_axon_ntff_profile_hook  # noqa: F401

            return  # already present and registered
        except ImportError:
            pass
        mod = types.ModuleType("antenv.axon_hooks")
        _holder = [None]
        mod.set_axon_ntff_profile_hook = lambda h: _holder.__setitem__(0, h)
        mod.get_axon_ntff_profile_hook = lambda: _holder[0]
        sys.modules["antenv.axon_hooks"] = mod
        antenv.axon_hooks = mod

        from trn_agent_boot.trn_boot import _ntff_profile_via_ctypes

        hook = _ntff_profile_via_ctypes("/opt/axon/libaxon_pjrt.so")
        mod.set_axon_ntff_profile_hook(hook)
    except Exception:
        pass


def kernel(**inputs):
    global LAST_RESULTS
    G = np.asarray(inputs["geneset_features"], dtype=np.float32)
    logits = np.asarray(inputs["attn_logits"], dtype=np.float32)
    flat_idx = np.asarray(inputs["flat_idx"]).astype(np.int64)
    seg = np.asarray(inputs["segment_ids"]).astype(np.int64)
    T = logits.shape[0]

    # Host-side layout prep: scatter exp(logits) into the sparse aggregation
    # matrix (member sets are sampled without replacement, so (idx, seg) pairs
    # are unique within a set and the fancy assignment is collision-free).
    e32 = np.exp(logits)
    W = np.zeros((NUM_GENESETS, NUM_SETS), dtype=ml_dtypes.bfloat16)
    W[flat_idx, seg] = e32.astype(ml_dtypes.bfloat16)

    # Padded per-set logit columns; device computes denominators from these.
    sizes = np.bincount(seg, minlength=NUM_SETS)
    starts = np.concatenate([[0], np.cumsum(sizes)[:-1]])
    pos = np.arange(T) - starts[seg]
    plogT = np.full((PAD_SLOTS, NUM_SETS), NEG_FILL, dtype=np.float32)
    plogT[pos, seg] = logits

    Gb = G.astype(ml_dtypes.bfloat16)

    GbT = np.ascontiguousarray(Gb.T)  # (8192, 1024)
    in_maps = []
    for c in range(N_CORES):
        bg, sg = divmod(c, SG)
        gt = GbT[:, bg * B_C : (bg + 1) * B_C].reshape(K_TILES, P, B_C)
        w = W[:, sg * S_C : (sg + 1) * S_C].reshape(K_TILES, P, S_C)
        gw = np.concatenate([gt, w], axis=2)  # (K_TILES, P, B_C + S_C)
        plog = np.ascontiguousarray(plogT[:, sg * S_C : (sg + 1) * S_C])
        in_maps.append({"gw": np.ascontiguousarray(gw), "plog": plog})

    from concourse.bass_utils import run_bass_kernel_spmd

    _ensure_ntff_hook()
    nc = _get_program()
    res = run_bass_kernel_spmd(nc, in_maps, core_ids=list(range(N_CORES)))
    LAST_RESULTS = res

    out = np.empty((BATCH, NUM_SETS), dtype=np.float32)
    for c in range(N_CORES):
        bg, sg = divmod(c, SG)
        out[bg * B_C : (bg + 1) * B_C, sg * S_C : (sg + 1) * S_C] = res.results[c][
            "out"
        ]
    return out
